# revision 13
# baseline (speedup 1.0000x reference)
"""Trainium2 Bass kernel for nn_Block2DGRU: LN -> dw3x3 conv -> bidirectional
minGRU -> MLP, data-parallel over batch (32 samples -> 8 cores x 4).

v2: fp8e4 DoubleRow matmuls (4x PE) for conv/GRU/MLP GEMMs, f16 I/O,
algebraic folds (conv bias -> GRU sigmoid bias; gamma2/beta2 -> p1w/pb1;
LN mean subtraction -> negated conv taps / extra GEMM K-chunk; residual
adds -> identity matmuls into PSUM; g = max(sigmoid(h), h+0.5)), and
engine balancing across PE/Act/DVE/Pool.

Layout: per-sample transposed [d, L] (channels on partitions). The minGRU
log-space Heinsen scan runs in linear space via DVE tensor_tensor_scan;
direction 2 scans backward through negative-stride APs.
"""
import numpy as np
import ml_dtypes
import concourse.bacc as bacc
import concourse.tile as tile
import concourse.mybir as mybir
from concourse.bass import AP
from concourse.bass_utils import run_bass_kernel_spmd

N_CORES = 8
NS = 4          # samples per core
DIM = 384
DI = 768        # minGRU inner dim
MLPD = 1536
L = 1024        # 32*32 flattened grid
GH = GW = 32
EPS = 1e-5
NC_D = 3        # input-channel chunks of 128
NC_H = 6        # hidden chunks (DI)
NC_M = 12       # mlp chunks (MLPD)
PADW = 34
PADN = PADW * PADW  # 1156

f32 = mybir.dt.float32
f16 = mybir.dt.float16
f8 = mybir.dt.float8e4
Alu = mybir.AluOpType
Act = mybir.ActivationFunctionType
DR = mybir.MatmulPerfMode.DoubleRow

F8 = ml_dtypes.float8_e4m3

# conv tap pairs (flat tap index t -> padded offset (t//3)*34 + t%3)
TAP_PAIRS = [(0, 1), (2, 3), (4, 5), (6, 7), (8, None)]
TAP_OFF = [(t // 3) * PADW + (t % 3) for t in range(9)]


def _win(tilap, base, dpair):
    """4D window AP [128, 2, 16, 32] over a padded [128, 1156] tile."""
    return AP(tilap.tensor, tilap.offset + base,
              [list(tilap.ap[0]), [dpair, 2], [PADW, 16], [1, 32]])


def build_nc(ns=NS, num_devices=N_CORES):
    nc = bacc.Bacc("TRN2", target_bir_lowering=False, debug=False,
                   num_devices=num_devices)

    # ---- DRAM I/O ----
    xT_d = nc.dram_tensor("xT", [ns, DIM, L], f16, kind="ExternalInput")
    g1w_d = nc.dram_tensor("g1w", [128, 4, 2 * DI], f8, kind="ExternalInput")
    g2w_d = nc.dram_tensor("g2w", [128, 4, 2 * DI], f8, kind="ExternalInput")
    g1o_d = nc.dram_tensor("g1o", [128, NC_H, DIM], f16, kind="ExternalInput")
    g2o_d = nc.dram_tensor("g2o", [128, NC_H, DIM], f16, kind="ExternalInput")
    p1w_d = nc.dram_tensor("p1w", [128, 4, MLPD], f8, kind="ExternalInput")
    p2w_d = nc.dram_tensor("p2w", [128, NC_M, DIM], f8, kind="ExternalInput")
    cdx_d = nc.dram_tensor("cdx", [NC_D, 128, 10, 2, 128], f8,
                           kind="ExternalInput")
    ident_d = nc.dram_tensor("ident", [128, 128], f16, kind="ExternalInput")
    ch_d = nc.dram_tensor("ch", [128, 12], f32, kind="ExternalInput")
    ch05_d = nc.dram_tensor("ch05", [128, 12], f32, kind="ExternalInput")
    cg_d = nc.dram_tensor("cg", [128, 12], f32, kind="ExternalInput")
    pb1_d = nc.dram_tensor("pb1", [128, NC_M], f32, kind="ExternalInput")
    pb2_d = nc.dram_tensor("pb2", [128, NC_D], f32, kind="ExternalInput")
    y_d = nc.dram_tensor("y", [ns, DIM, L], f16, kind="ExternalOutput")

    with tile.TileContext(nc) as tc:
        with tc.tile_pool(name="wp", bufs=1) as wp, \
             tc.tile_pool(name="sb", bufs=1) as sb, \
             tc.tile_pool(name="ps", bufs=1, space="PSUM") as ps:

            # ---- persistent weights ----
            ones = wp.tile([128, 128], f16, tag="ones", name="ones")
            nc.vector.memset(ones[:], 1.0)
            epsc = wp.tile([128, 1], f32, tag="epsc", name="epsc")
            nc.vector.memset(epsc[:], EPS)
            g1w = wp.tile([128, 4, 2 * DI], f8, tag="g1w", name="g1w")
            g2w = wp.tile([128, 4, 2 * DI], f8, tag="g2w", name="g2w")
            g1o = wp.tile([128, NC_H, DIM], f16, tag="g1o", name="g1o")
            g2o = wp.tile([128, NC_H, DIM], f16, tag="g2o", name="g2o")
            p1w = wp.tile([128, 4, MLPD], f8, tag="p1w", name="p1w")
            p2w = wp.tile([128, NC_M, DIM], f8, tag="p2w", name="p2w")
            cdx = [wp.tile([128, 10, 2, 128], f8, tag=f"cdx{c}",
                           name=f"cdx{c}") for c in range(NC_D)]
            ident = wp.tile([128, 128], f16, tag="ident", name="ident")
            ch = wp.tile([128, 12], f32, tag="ch", name="ch")
            ch05 = wp.tile([128, 12], f32, tag="ch05", name="ch05")
            cg = wp.tile([128, 12], f32, tag="cg", name="cg")
            pb1 = wp.tile([128, NC_M], f32, tag="pb1", name="pb1")
            pb2 = wp.tile([128, NC_D], f32, tag="pb2", name="pb2")

            for t, d in [(g1w, g1w_d), (g2w, g2w_d), (g1o, g1o_d),
                         (g2o, g2o_d), (p1w, p1w_d), (p2w, p2w_d),
                         (ident, ident_d), (ch, ch_d), (ch05, ch05_d),
                         (cg, cg_d), (pb1, pb1_d), (pb2, pb2_d)]:
                nc.sync.dma_start(t[:], d.ap()[:])
            for c in range(NC_D):
                nc.sync.dma_start(cdx[c][:], cdx_d.ap()[c])

            # padded LN1-output / pm tiles (borders stay zero; interior
            # rewritten per sample). Two explicit buffers for pipelining.
            xnp = [[wp.tile([128, PADN], f8, tag=f"xnp{b}_{c}",
                            name=f"xnp{b}_{c}") for c in range(NC_D)]
                   for b in range(2)]
            pmp = [wp.tile([128, PADN], f8, tag=f"pmp{b}", name=f"pmp{b}")
                   for b in range(2)]
            for b in range(2):
                for c in range(NC_D):
                    nc.vector.memset(xnp[b][c][:], 0.0)
                nc.vector.memset(pmp[b][:], 0.0)

            r32 = lambda ap: ap.rearrange("p (a b) -> p a b", a=GH)

            def layer_norm(src, tag):
                """src: [128, 3, L] f16 tile. Returns (mu f16, rstd f16)."""
                S = ps.tile([128, L], f32, tag="bank", name=f"S_{tag}", bufs=4)
                SS = ps.tile([128, L], f32, tag="bank", name=f"SS_{tag}",
                             bufs=4)
                for c in range(NC_D):
                    sq = sb.tile([128, L], f16, tag="sq", name=f"sq{c}_{tag}",
                                 bufs=2)
                    nc.vector.tensor_tensor(sq[:], src[:, c, :], src[:, c, :],
                                            op=Alu.mult)
                    for h in range(2):
                        sl = slice(h * 512, (h + 1) * 512)
                        nc.tensor.matmul(S[:, sl], ones[:], src[:, c, sl],
                                         start=(c == 0), stop=(c == NC_D - 1))
                        nc.tensor.matmul(SS[:, sl], ones[:], sq[:, sl],
                                         start=(c == 0), stop=(c == NC_D - 1))
                mu = sb.tile([128, L], f16, tag="mu", name=f"mu_{tag}", bufs=2)
                with nc.allow_low_precision(reason="mu f16 ok at 2e-2 tol"):
                    nc.scalar.activation(mu[:], S[:], Act.Copy,
                                         scale=1.0 / DIM)
                m2 = sb.tile([128, L], f16, tag="m2", name=f"m2_{tag}", bufs=2)
                nc.vector.tensor_tensor(m2[:], mu[:], mu[:], op=Alu.mult)
                v = sb.tile([128, L], f32, tag="v", name=f"v_{tag}", bufs=2)
                nc.vector.scalar_tensor_tensor(v[:], SS[:], 1.0 / DIM, m2[:],
                                               op0=Alu.mult, op1=Alu.subtract)
                sd = sb.tile([128, L], f32, tag="sd", name=f"sd_{tag}", bufs=2)
                nc.scalar.activation(sd[:], v[:], Act.Sqrt, bias=epsc[:])
                rstd = sb.tile([128, L], f16, tag="rstd", name=f"rstd_{tag}",
                               bufs=2)
                with nc.allow_low_precision(reason="rstd f16 ok at 2e-2 tol"):
                    nc.vector.reciprocal(rstd[:], sd[:])
                return mu, rstd

            for s in range(ns):
                xb = xnp[s % 2]
                pb = pmp[s % 2]
                # ---- load sample ----
                xh = sb.tile([128, NC_D, L], f16, tag="xh", name=f"xh{s}",
                             bufs=2)
                for c in range(NC_D):
                    nc.sync.dma_start(xh[:, c, :],
                                      xT_d.ap()[s, c * 128:(c + 1) * 128, :])

                # ---- LN1 ----
                mu1, rstd1 = layer_norm(xh, f"1_{s}")
                for c in range(NC_D):
                    xi = xb[c][:].rearrange("p (a b) -> p a b", a=PADW)
                    nc.gpsimd.tensor_tensor(
                        xi[:, 1:GH + 1, 1:GW + 1], r32(xh[:, c, :]),
                        r32(rstd1[:]), op=Alu.mult)
                pi = pb[:].rearrange("p (a b) -> p a b", a=PADW)
                nc.gpsimd.tensor_tensor(
                    pi[:, 1:GH + 1, 1:GW + 1], r32(mu1[:]), r32(rstd1[:]),
                    op=Alu.mult)

                # ---- depthwise conv (DoubleRow tap pairs, pm negated) ----
                xs = sb.tile([128, 4, L], f8, tag="xs", name=f"xs{s}", bufs=2)
                nc.vector.memset(xs[:, 3, :], 0.0)
                for c in range(NC_D):
                    cv = ps.tile([128, L], f32, tag="bank", name=f"cv{c}_{s}",
                                 bufs=4)
                    for h in range(2):
                        sl = slice(h * 512, (h + 1) * 512)
                        for q in range(10):
                            t0, t1 = TAP_PAIRS[q % 5]
                            base = TAP_OFF[t0] + PADW * 16 * h
                            dp = (TAP_OFF[t1] - TAP_OFF[t0]) if t1 is not None else -1
                            src = xb[c] if q < 5 else pb
                            rhs = _win(src[:], base, dp)
                            nc.tensor.matmul(cv[:, sl], cdx[c][:, q], rhs,
                                             start=(q == 0), stop=(q == 9),
                                             perf_mode=DR)
                    nc.vector.tensor_copy(xs[:, c, :], cv[:])

                # ---- bidirectional minGRU ----
                hhs = []
                for d in range(2):
                    gw = g1w if d == 0 else g2w
                    hh = sb.tile([128, NC_H, L], f16, tag=f"hh{d}",
                                 name=f"hh{d}_{s}", bufs=1)
                    hhs.append(hh)
                    for j in range(NC_H):
                        H = ps.tile([128, L], f32, tag="bank",
                                    name=f"H{d}{j}_{s}", bufs=4)
                        G = ps.tile([128, L], f32, tag="bank",
                                    name=f"G{d}{j}_{s}", bufs=4)
                        for h in range(2):
                            sl = slice(h * 512, (h + 1) * 512)
                            for i in range(2):
                                rhs = xs[:, 2 * i:2 * i + 2, sl]
                                nc.tensor.matmul(
                                    H[:, sl],
                                    gw[:, 2 * i:2 * i + 2,
                                       j * 128:(j + 1) * 128],
                                    rhs, start=(i == 0), stop=(i == 1),
                                    perf_mode=DR)
                                nc.tensor.matmul(
                                    G[:, sl],
                                    gw[:, 2 * i:2 * i + 2,
                                       (NC_H + j) * 128:(NC_H + j + 1) * 128],
                                    rhs, start=(i == 0), stop=(i == 1),
                                    perf_mode=DR)
                        jc = d * 6 + j
                        z = sb.tile([128, L], f16, tag="z", name="z", bufs=2)
                        nc.scalar.activation(z[:], G[:], Act.Sigmoid,
                                             bias=cg[:, jc:jc + 1])
                        sh = sb.tile([128, L], f16, tag="sh", name="sh",
                                     bufs=2)
                        nc.scalar.activation(sh[:], H[:], Act.Sigmoid,
                                             bias=ch[:, jc:jc + 1])
                        g = sb.tile([128, L], f16, tag="g", name="g", bufs=2)
                        nc.vector.scalar_tensor_tensor(
                            g[:], H[:], ch05[:, jc:jc + 1], sh[:],
                            op0=Alu.add, op1=Alu.max)
                        a = sb.tile([128, L], f16, tag="a", name="a", bufs=2)
                        nc.vector.tensor_scalar(a[:], z[:], -1.0, 1.0,
                                                op0=Alu.mult, op1=Alu.add)
                        b = sb.tile([128, L], f16, tag="b", name="b", bufs=2)
                        nc.gpsimd.tensor_tensor(b[:], z[:], g[:], op=Alu.mult)
                        if d == 0:
                            nc.vector.tensor_tensor_scan(
                                hh[:, j], a[:], b[:], 0.0,
                                op0=Alu.mult, op1=Alu.add)
                        else:
                            nc.vector.tensor_tensor_scan(
                                hh[:, j][:, ::-1], a[:, ::-1], b[:, ::-1],
                                0.0, op0=Alu.mult, op1=Alu.add)
                # out-projection (both dirs) + residual via identity matmul
                yt = sb.tile([128, NC_D, L], f16, tag="yt", name=f"yt{s}",
                             bufs=2)
                for m in range(NC_D):
                    q = ps.tile([128, L], f32, tag="bank", name=f"q{m}_{s}",
                                bufs=4)
                    for h in range(2):
                        sl = slice(h * 512, (h + 1) * 512)
                        for d, go in ((0, g1o), (1, g2o)):
                            for j in range(NC_H):
                                nc.tensor.matmul(
                                    q[:, sl],
                                    go[:, j, m * 128:(m + 1) * 128],
                                    hhs[d][:, j, sl],
                                    start=(d == 0 and j == 0), stop=False)
                        nc.tensor.matmul(q[:, sl], ident[:], xh[:, m, sl],
                                         start=False, stop=True)
                    nc.vector.tensor_copy(yt[:, m, :], q[:])

                # ---- LN2 + MLP ----
                mu2, rstd2 = layer_norm(yt, f"2_{s}")
                yn = sb.tile([128, 4, L], f8, tag="yn", name=f"yn{s}", bufs=2)
                for c in range(NC_D):
                    nc.gpsimd.tensor_tensor(yn[:, c, :], yt[:, c, :],
                                            rstd2[:], op=Alu.mult)
                nc.gpsimd.tensor_tensor(yn[:, 3, :], mu2[:], rstd2[:],
                                        op=Alu.mult)
                yh = sb.tile([128, NC_M, L], f8, tag="yh", name=f"yh{s}",
                             bufs=1)
                for j in range(NC_M):
                    pp = ps.tile([128, L], f32, tag="bank", name=f"pp{j}_{s}",
                                 bufs=4)
                    for h in range(2):
                        sl = slice(h * 512, (h + 1) * 512)
                        for i in range(2):
                            nc.tensor.matmul(
                                pp[:, sl],
                                p1w[:, 2 * i:2 * i + 2,
                                    j * 128:(j + 1) * 128],
                                yn[:, 2 * i:2 * i + 2, sl],
                                start=(i == 0), stop=(i == 1), perf_mode=DR)
                    nc.scalar.activation(yh[:, j], pp[:], Act.Gelu,
                                         bias=pb1[:, j:j + 1])
                yo = sb.tile([128, NC_D, L], f16, tag="yo", name=f"yo{s}",
                             bufs=2)
                for m in range(NC_D):
                    q2 = ps.tile([128, L], f32, tag="bank", name=f"q2{m}_{s}",
                                 bufs=4)
                    for h in range(2):
                        sl = slice(h * 512, (h + 1) * 512)
                        for i in range(NC_M // 2):
                            nc.tensor.matmul(
                                q2[:, sl],
                                p2w[:, 2 * i:2 * i + 2,
                                    m * 128:(m + 1) * 128],
                                yh[:, 2 * i:2 * i + 2, sl],
                                start=(i == 0), stop=(i == NC_M // 2 - 1),
                                perf_mode=DR)
                    nc.vector.scalar_tensor_tensor(
                        yo[:, m, :], q2[:], pb2[:, m:m + 1], yt[:, m, :],
                        op0=Alu.add, op1=Alu.add)
                for c in range(NC_D):
                    nc.sync.dma_start(y_d.ap()[s, c * 128:(c + 1) * 128, :],
                                      yo[:, c, :])

    nc.compile()
    return nc


_NC_CACHE = {}


def _get_nc(ns=NS, num_devices=N_CORES):
    key = (ns, num_devices)
    if key not in _NC_CACHE:
        _NC_CACHE[key] = build_nc(ns, num_devices)
    return _NC_CACHE[key]


def _kchunk(w, nchunks):
    """[K, M] -> [128, nchunks, M] fp8, zero-padding K to nchunks*128."""
    K, M = w.shape
    out = np.zeros((128, nchunks, M), np.float32)
    for k in range(nchunks):
        lo = k * 128
        hi = min(K, lo + 128)
        if lo < K:
            out[:hi - lo, k, :] = w[lo:hi, :]
    return out.astype(F8)


def _kchunk16(w, nchunks):
    K, M = w.shape
    out = np.zeros((128, nchunks, M), np.float32)
    for k in range(nchunks):
        lo = k * 128
        hi = min(K, lo + 128)
        if lo < K:
            out[:hi - lo, k, :] = w[lo:hi, :]
    return out.astype(np.float16)


def make_weight_maps(gamma1, beta1, dwc_w, dwc_b, gru1_w, gru1_out,
                     gru2_w, gru2_out, gamma2, beta2, p1_w, p1_b, p2_w, p2_b):
    f = np.float32
    g1 = np.asarray(gamma1, f)
    dwc = np.asarray(dwc_w, f).reshape(DIM, 9)
    # conv taps: gamma1 folded in; pairs 5-9 are negated pm taps
    cdx = np.zeros((NC_D, 128, 10, 2, 128), f)
    for c in range(NC_D):
        wg = dwc[c * 128:(c + 1) * 128] * g1[c * 128:(c + 1) * 128, None]
        for q in range(5):
            t0, t1 = TAP_PAIRS[q]
            np.einsum('ii->i', cdx[c, :, q, 0])[:] = wg[:, t0]
            np.einsum('ii->i', cdx[c, :, q + 5, 0])[:] = -wg[:, t0]
            if t1 is not None:
                np.einsum('ii->i', cdx[c, :, q, 1])[:] = wg[:, t1]
                np.einsum('ii->i', cdx[c, :, q + 5, 1])[:] = -wg[:, t1]

    # conv bias -> GRU sigmoid bias: hgb_d = dwc_b @ gru_w_d
    w1 = np.asarray(gru1_w, f)
    w2 = np.asarray(gru2_w, f)
    db = np.asarray(dwc_b, f)
    hgb1 = db @ w1
    hgb2 = db @ w2
    colmat = lambda v, n: np.ascontiguousarray(v.reshape(n, 128).T)
    ch = np.concatenate([colmat(hgb1[:DI], 6), colmat(hgb2[:DI], 6)], axis=1)
    cg = np.concatenate([colmat(hgb1[DI:], 6), colmat(hgb2[DI:], 6)], axis=1)

    # gamma2/beta2 fold into p1w/pb1; pm2 row in K-chunk 3
    p1e = np.asarray(p1_w, f) * np.asarray(gamma2, f)[:, None]
    pb1e = np.asarray(p1_b, f) + np.asarray(beta2, f) @ np.asarray(p1_w, f)
    p1c = _kchunk(p1e, 4).astype(np.float32)
    p1c[0, 3, :] = -p1e.sum(axis=0)
    return dict(
        g1w=_kchunk(w1, 4), g2w=_kchunk(w2, 4),
        g1o=_kchunk16(np.asarray(gru1_out, f), NC_H),
        g2o=_kchunk16(np.asarray(gru2_out, f), NC_H),
        p1w=p1c.astype(F8), p2w=_kchunk(np.asarray(p2_w, f), NC_M),
        cdx=cdx.astype(F8),
        ident=np.eye(128, dtype=np.float16),
        ch=ch, ch05=ch + 0.5, cg=cg,
        pb1=colmat(pb1e, NC_M), pb2=colmat(np.asarray(p2_b, f), NC_D),
    )


def kernel(x, gamma1, beta1, dwc_w, dwc_b, gru1_w, gru1_out, gru2_w, gru2_out,
           gamma2, beta2, p1_w, p1_b, p2_w, p2_b, h, w):
    assert np.allclose(np.asarray(beta1), 0.0), "beta1 fold not implemented"
    x = np.asarray(x, np.float32)
    nc = _get_nc()
    xT = np.ascontiguousarray(x.transpose(0, 2, 1)).astype(np.float16)
    wmap = make_weight_maps(gamma1, beta1, dwc_w, dwc_b, gru1_w, gru1_out,
                            gru2_w, gru2_out, gamma2, beta2, p1_w, p1_b,
                            p2_w, p2_b)
    in_maps = []
    for i in range(N_CORES):
        m = dict(wmap)
        m["xT"] = xT[i * NS:(i + 1) * NS]
        in_maps.append(m)
    res = run_bass_kernel_spmd(nc, in_maps, list(range(N_CORES)))
    yT = np.concatenate([res.results[i]["y"] for i in range(N_CORES)], axis=0)
    return np.ascontiguousarray(
        yT.astype(np.float32).transpose(0, 2, 1))


# revision 22
# speedup vs baseline: 2.4289x; 2.4289x over previous
"""Trainium2 Bass kernel for nn_Block2DGRU: LN -> dw3x3 conv -> bidirectional
minGRU -> MLP, data-parallel over batch (32 samples -> 8 cores x 4).

v2: fp8e4 DoubleRow matmuls (4x PE) for conv/GRU/MLP GEMMs, f16 I/O,
algebraic folds (conv bias -> GRU sigmoid bias; gamma2/beta2 -> p1w/pb1;
LN mean subtraction -> negated conv taps / extra GEMM K-chunk; residual
adds -> identity matmuls into PSUM; g = max(sigmoid(h), h+0.5)), and
engine balancing across PE/Act/DVE/Pool.

Layout: per-sample transposed [d, L] (channels on partitions). The minGRU
log-space Heinsen scan runs in linear space via DVE tensor_tensor_scan;
direction 2 scans backward through negative-stride APs.
"""
import numpy as np
import ml_dtypes
import concourse.bacc as bacc
import concourse.tile as tile
import concourse.mybir as mybir
from concourse.bass import AP
from concourse.bass_utils import run_bass_kernel_spmd

N_CORES = 8
NS = 4          # samples per core
DIM = 384
DI = 768        # minGRU inner dim
MLPD = 1536
L = 1024        # 32*32 flattened grid
GH = GW = 32
EPS = 1e-5
NC_D = 3        # input-channel chunks of 128
NC_H = 6        # hidden chunks (DI)
NC_M = 12       # mlp chunks (MLPD)
PADW = 34
PADN = PADW * PADW  # 1156

f32 = mybir.dt.float32
f16 = mybir.dt.float16
f8 = mybir.dt.float8e4
Alu = mybir.AluOpType
Act = mybir.ActivationFunctionType
DR = mybir.MatmulPerfMode.DoubleRow

F8 = ml_dtypes.float8_e4m3

# conv tap pairs (flat tap index t -> padded offset (t//3)*34 + t%3)
TAP_PAIRS = [(0, 1), (2, 3), (4, 5), (6, 7), (8, None)]
TAP_OFF = [(t // 3) * PADW + (t % 3) for t in range(9)]


def _win(tilap, base, dpair):
    """4D window AP [128, 2, 16, 32] over a padded [128, 1156] tile."""
    return AP(tilap.tensor, tilap.offset + base,
              [list(tilap.ap[0]), [dpair, 2], [PADW, 16], [1, 32]])


def build_nc(ns=NS, num_devices=N_CORES):
    nc = bacc.Bacc("TRN2", target_bir_lowering=False, debug=False,
                   num_devices=num_devices)

    # ---- DRAM I/O ----
    xT_d = nc.dram_tensor("xT", [ns, DIM, L], f16, kind="ExternalInput")
    g1w_d = nc.dram_tensor("g1w", [128, 4, 2 * DI], f8, kind="ExternalInput")
    g2w_d = nc.dram_tensor("g2w", [128, 4, 2 * DI], f8, kind="ExternalInput")
    g1o_d = nc.dram_tensor("g1o", [128, NC_H, DIM], f16, kind="ExternalInput")
    g2o_d = nc.dram_tensor("g2o", [128, NC_H, DIM], f16, kind="ExternalInput")
    p1w_d = nc.dram_tensor("p1w", [128, 4, MLPD], f8, kind="ExternalInput")
    p2w_d = nc.dram_tensor("p2w", [128, NC_M, DIM], f8, kind="ExternalInput")
    cdx_d = nc.dram_tensor("cdx", [NC_D, 128, 10, 2, 128], f8,
                           kind="ExternalInput")
    ident_d = nc.dram_tensor("ident", [128, 128], f16, kind="ExternalInput")
    pb1_d = nc.dram_tensor("pb1", [128, NC_M], f32, kind="ExternalInput")
    pb2_d = nc.dram_tensor("pb2", [128, NC_D], f32, kind="ExternalInput")
    y_d = nc.dram_tensor("y", [ns, DIM, L], f16, kind="ExternalOutput")

    with tile.TileContext(nc) as tc:
        with tc.tile_pool(name="wp", bufs=1) as wp, \
             tc.tile_pool(name="sb", bufs=1) as sb, \
             tc.tile_pool(name="ps", bufs=1, space="PSUM") as ps:

            # ---- persistent weights ----
            ones = wp.tile([128, 128], f16, tag="ones", name="ones")
            nc.vector.memset(ones[:], 1.0)
            epsc = wp.tile([128, 1], f32, tag="epsc", name="epsc")
            nc.vector.memset(epsc[:], EPS)
            g1w = wp.tile([128, 4, 2 * DI], f8, tag="g1w", name="g1w")
            g2w = wp.tile([128, 4, 2 * DI], f8, tag="g2w", name="g2w")
            g1o = wp.tile([128, NC_H, DIM], f16, tag="g1o", name="g1o")
            g2o = wp.tile([128, NC_H, DIM], f16, tag="g2o", name="g2o")
            p1w = wp.tile([128, 4, MLPD], f8, tag="p1w", name="p1w")
            p2w = wp.tile([128, NC_M, DIM], f8, tag="p2w", name="p2w")
            cdx = [wp.tile([128, 10, 2, 128], f8, tag=f"cdx{c}",
                           name=f"cdx{c}") for c in range(NC_D)]
            ident = wp.tile([128, 128], f16, tag="ident", name="ident")
            pb1 = wp.tile([128, NC_M], f32, tag="pb1", name="pb1")
            pb2 = wp.tile([128, NC_D], f32, tag="pb2", name="pb2")

            for t, d in [(g1w, g1w_d), (g2w, g2w_d), (g1o, g1o_d),
                         (g2o, g2o_d), (p1w, p1w_d), (p2w, p2w_d),
                         (ident, ident_d), (pb1, pb1_d), (pb2, pb2_d)]:
                nc.sync.dma_start(t[:], d.ap()[:])
            for c in range(NC_D):
                nc.sync.dma_start(cdx[c][:], cdx_d.ap()[c])

            # padded LN1-output / pm tiles (borders stay zero; interior
            # rewritten per sample). Two explicit buffers for pipelining.
            xnp = [[wp.tile([128, PADN], f8, tag=f"xnp{b}_{c}",
                            name=f"xnp{b}_{c}") for c in range(NC_D)]
                   for b in range(2)]
            pmp = [wp.tile([128, PADN], f8, tag=f"pmp{b}", name=f"pmp{b}")
                   for b in range(2)]
            for b in range(2):
                for c in range(NC_D):
                    nc.gpsimd.memset(xnp[b][c][:], 0.0)
                nc.gpsimd.memset(pmp[b][:], 0.0)

            r32 = lambda ap: ap.rearrange("p (a b) -> p a b", a=GH)

            def layer_norm(src, tag):
                """src: [128, 3, L] f16 tile. Returns (mu f16, rstd f16)."""
                S = ps.tile([128, L], f32, tag="bank", name=f"S_{tag}", bufs=4)
                SS = ps.tile([128, L], f32, tag="bank", name=f"SS_{tag}",
                             bufs=4)
                for c in range(NC_D):
                    sq = sb.tile([128, L], f16, tag="sq", name=f"sq{c}_{tag}",
                                 bufs=2)
                    nc.vector.tensor_tensor(sq[:], src[:, c, :], src[:, c, :],
                                            op=Alu.mult)
                    for h in range(2):
                        sl = slice(h * 512, (h + 1) * 512)
                        nc.tensor.matmul(S[:, sl], ones[:], src[:, c, sl],
                                         start=(c == 0), stop=(c == NC_D - 1))
                        nc.tensor.matmul(SS[:, sl], ones[:], sq[:, sl],
                                         start=(c == 0), stop=(c == NC_D - 1))
                mu = sb.tile([128, L], f16, tag="mu", name=f"mu_{tag}", bufs=2)
                with nc.allow_low_precision(reason="mu f16 ok at 2e-2 tol"):
                    nc.scalar.activation(mu[:], S[:], Act.Copy,
                                         scale=1.0 / DIM)
                m2 = sb.tile([128, L], f16, tag="m2", name=f"m2_{tag}", bufs=2)
                with nc.allow_low_precision(reason="m2 f16 ok at 2e-2 tol"):
                    nc.scalar.activation(m2[:], mu[:], Act.Square)
                v = sb.tile([128, L], f32, tag="v", name=f"v_{tag}", bufs=2)
                nc.vector.scalar_tensor_tensor(v[:], SS[:], 1.0 / DIM, m2[:],
                                               op0=Alu.mult, op1=Alu.subtract)
                sd = sb.tile([128, L], f32, tag="sd", name=f"sd_{tag}", bufs=2)
                nc.scalar.activation(sd[:], v[:], Act.Sqrt, bias=epsc[:])
                rstd = sb.tile([128, L], f16, tag="rstd", name=f"rstd_{tag}",
                               bufs=2)
                with nc.allow_low_precision(reason="rstd f16 ok at 2e-2 tol"):
                    nc.vector.reciprocal(rstd[:], sd[:])
                return mu, rstd

            # per-sample state carried between pipeline stages
            st = [dict() for _ in range(ns)]

            def stage_pre(s):
                """DMA in, LN1, padded applies, conv, xs."""
                xb, pb = xnp[s % 2], pmp[s % 2]
                xh = sb.tile([128, NC_D, L], f16, tag="xh", name=f"xh{s}",
                             bufs=2)
                for c in range(NC_D):
                    nc.sync.dma_start(xh[:, c, :],
                                      xT_d.ap()[s, c * 128:(c + 1) * 128, :])
                mu1, rstd1 = layer_norm(xh, f"1_{s}")
                pi = pb[:].rearrange("p (a b) -> p a b", a=PADW)
                nc.gpsimd.tensor_tensor(pi[:, 1:GH + 1, 1:GW + 1],
                                        r32(mu1[:]), r32(rstd1[:]),
                                        op=Alu.mult)
                xs = sb.tile([128, 4, L], f8, tag="xs", name=f"xs{s}", bufs=2)
                nc.gpsimd.memset(xs[:, 3, :], 1.0)
                for c in range(NC_D):
                    xi = xb[c][:].rearrange("p (a b) -> p a b", a=PADW)
                    eng = nc.vector if c != 1 else nc.gpsimd
                    eng.tensor_tensor(xi[:, 1:GH + 1, 1:GW + 1],
                                      r32(xh[:, c, :]), r32(rstd1[:]),
                                      op=Alu.mult)
                    cv = ps.tile([128, L], f32, tag="bank", name=f"cv{c}_{s}",
                                 bufs=4)
                    for h in range(2):
                        sl = slice(h * 512, (h + 1) * 512)
                        for q in range(10):
                            t0, t1 = TAP_PAIRS[q % 5]
                            base = TAP_OFF[t0] + PADW * 16 * h
                            dp = (TAP_OFF[t1] - TAP_OFF[t0]) if t1 is not None else -1
                            srcp = xb[c] if q < 5 else pb
                            rhs = _win(srcp[:], base, dp)
                            nc.tensor.matmul(cv[:, sl], cdx[c][:, q], rhs,
                                             start=(q == 0), stop=(q == 9),
                                             perf_mode=DR)
                    nc.vector.tensor_copy(xs[:, c, :], cv[:])
                st[s]["xh"] = xh
                st[s]["xs"] = xs

            def stage_mid(s, filler=iter(())):
                """Bidirectional minGRU inner: H/G GEMMs, gates, scans."""
                xs = st[s]["xs"]
                hhs = []
                scan_q = []

                def drain_scan():
                    dd, jj, aa, bb = scan_q.pop(0)
                    if dd == 0:
                        nc.vector.tensor_tensor_scan(
                            hhs[0][:, jj], aa[:], bb[:], 0.0,
                            op0=Alu.mult, op1=Alu.add)
                    else:
                        nc.vector.tensor_tensor_scan(
                            hhs[1][:, jj][:, ::-1], aa[:, ::-1], bb[:, ::-1],
                            0.0, op0=Alu.mult, op1=Alu.add)

                for d in range(2):
                    gw = g1w if d == 0 else g2w
                    hh = sb.tile([128, NC_H, L], f16, tag=f"hh{d}",
                                 name=f"hh{d}_{s}", bufs=1)
                    hhs.append(hh)
                    for j in range(NC_H):
                        H = ps.tile([128, L], f32, tag="bank",
                                    name=f"H{d}{j}_{s}", bufs=4)
                        G = ps.tile([128, L], f32, tag="bank",
                                    name=f"G{d}{j}_{s}", bufs=4)
                        for P, jw in ((H, j), (G, NC_H + j)):
                            for h in range(2):
                                sl = slice(h * 512, (h + 1) * 512)
                                for i in range(2):
                                    nc.tensor.matmul(
                                        P[:, sl],
                                        gw[:, 2 * i:2 * i + 2,
                                           jw * 128:(jw + 1) * 128],
                                        xs[:, 2 * i:2 * i + 2, sl],
                                        start=(i == 0), stop=(i == 1),
                                        perf_mode=DR)
                        z = sb.tile([128, L], f16, tag="z", name="z", bufs=3)
                        nc.scalar.activation(z[:], G[:], Act.Sigmoid)
                        sh = sb.tile([128, L], f16, tag="sh", name="sh",
                                     bufs=3)
                        nc.scalar.activation(sh[:], H[:], Act.Sigmoid)
                        g = sb.tile([128, L], f16, tag="g", name="g", bufs=3)
                        nc.vector.scalar_tensor_tensor(
                            g[:], H[:], 0.5, sh[:], op0=Alu.add, op1=Alu.max)
                        a = sb.tile([128, L], f16, tag="a", name="a", bufs=3)
                        nc.vector.tensor_scalar(a[:], z[:], -1.0, 1.0,
                                                op0=Alu.mult, op1=Alu.add)
                        b = sb.tile([128, L], f16, tag="b", name="b", bufs=3)
                        nc.gpsimd.tensor_tensor(b[:], z[:], g[:],
                                                op=Alu.mult)
                        scan_q.append((d, j, a, b))
                        if len(scan_q) > 2:
                            drain_scan()
                        next(filler, None)
                while scan_q:
                    drain_scan()
                st[s]["hhs"] = hhs

            def stage_post_a(s):
                """Out-projection + residual, LN2, yn."""
                xh, hhs = st[s]["xh"], st[s]["hhs"]
                yt = sb.tile([128, NC_D, L], f16, tag="yt", name=f"yt{s}",
                             bufs=2)
                for m in range(NC_D):
                    q = ps.tile([128, L], f32, tag="bank", name=f"q{m}_{s}",
                                bufs=4)
                    for h in range(2):
                        sl = slice(h * 512, (h + 1) * 512)
                        for d, go in ((0, g1o), (1, g2o)):
                            for j in range(NC_H):
                                nc.tensor.matmul(
                                    q[:, sl],
                                    go[:, j, m * 128:(m + 1) * 128],
                                    hhs[d][:, j, sl],
                                    start=(d == 0 and j == 0), stop=False)
                        nc.tensor.matmul(q[:, sl], ident[:], xh[:, m, sl],
                                         start=False, stop=True)
                    nc.vector.tensor_copy(yt[:, m, :], q[:])

                mu2, rstd2 = layer_norm(yt, f"2_{s}")
                yn = sb.tile([128, 4, L], f8, tag="yn", name=f"yn{s}", bufs=2)
                for c in range(NC_D):
                    eng = nc.vector if c != 1 else nc.gpsimd
                    eng.tensor_tensor(yn[:, c, :], yt[:, c, :], rstd2[:],
                                      op=Alu.mult)
                nc.gpsimd.tensor_tensor(yn[:, 3, :], mu2[:], rstd2[:],
                                        op=Alu.mult)
                st[s]["yt"] = yt
                st[s]["yn"] = yn

            def stage_post_b(s):
                """MLP + DMA out (generator: yields between units so the
                caller can interleave with other stages)."""
                yt, yn = st[s]["yt"], st[s]["yn"]
                yh = sb.tile([128, NC_M, L], f8, tag="yh", name=f"yh{s}",
                             bufs=1)
                for j in range(NC_M):
                    pp = ps.tile([128, L], f32, tag="bank", name=f"pp{j}_{s}",
                                 bufs=4)
                    for h in range(2):
                        sl = slice(h * 512, (h + 1) * 512)
                        for i in range(2):
                            nc.tensor.matmul(
                                pp[:, sl],
                                p1w[:, 2 * i:2 * i + 2,
                                    j * 128:(j + 1) * 128],
                                yn[:, 2 * i:2 * i + 2, sl],
                                start=(i == 0), stop=(i == 1), perf_mode=DR)
                    nc.scalar.activation(yh[:, j], pp[:], Act.Gelu,
                                         bias=pb1[:, j:j + 1])
                    yield
                yo = sb.tile([128, NC_D, L], f16, tag="yo", name=f"yo{s}",
                             bufs=2)
                for m in range(NC_D):
                    q2 = ps.tile([128, L], f32, tag="bank", name=f"q2{m}_{s}",
                                 bufs=4)
                    for h in range(2):
                        sl = slice(h * 512, (h + 1) * 512)
                        for i in range(NC_M // 2):
                            nc.tensor.matmul(
                                q2[:, sl],
                                p2w[:, 2 * i:2 * i + 2,
                                    m * 128:(m + 1) * 128],
                                yh[:, 2 * i:2 * i + 2, sl],
                                start=(i == 0), stop=(i == NC_M // 2 - 1),
                                perf_mode=DR)
                    nc.vector.scalar_tensor_tensor(
                        yo[:, m, :], q2[:], pb2[:, m:m + 1], yt[:, m, :],
                        op0=Alu.add, op1=Alu.add)
                    yield
                for c in range(NC_D):
                    nc.sync.dma_start(y_d.ap()[s, c * 128:(c + 1) * 128, :],
                                      yo[:, c, :])
                yield

            # software-pipelined emission: each engine queue interleaves
            # adjacent samples so in-order engines never drain
            stage_pre(0)
            for s in range(ns):
                stage_mid(s)
                stage_post_a(s)
                if s + 1 < ns:
                    stage_pre(s + 1)
                for _ in stage_post_b(s):
                    pass

    nc.compile()
    return nc


_NC_CACHE = {}


def _get_nc(ns=NS, num_devices=N_CORES):
    key = (ns, num_devices)
    if key not in _NC_CACHE:
        _NC_CACHE[key] = build_nc(ns, num_devices)
    return _NC_CACHE[key]


def _kchunk(w, nchunks):
    """[K, M] -> [128, nchunks, M] fp8, zero-padding K to nchunks*128."""
    K, M = w.shape
    out = np.zeros((128, nchunks, M), np.float32)
    for k in range(nchunks):
        lo = k * 128
        hi = min(K, lo + 128)
        if lo < K:
            out[:hi - lo, k, :] = w[lo:hi, :]
    return out.astype(F8)


def _kchunk16(w, nchunks):
    K, M = w.shape
    out = np.zeros((128, nchunks, M), np.float32)
    for k in range(nchunks):
        lo = k * 128
        hi = min(K, lo + 128)
        if lo < K:
            out[:hi - lo, k, :] = w[lo:hi, :]
    return out.astype(np.float16)


def make_weight_maps(gamma1, beta1, dwc_w, dwc_b, gru1_w, gru1_out,
                     gru2_w, gru2_out, gamma2, beta2, p1_w, p1_b, p2_w, p2_b):
    f = np.float32
    g1 = np.asarray(gamma1, f)
    dwc = np.asarray(dwc_w, f).reshape(DIM, 9)
    # conv taps: gamma1 folded in; pairs 5-9 are negated pm taps
    cdx = np.zeros((NC_D, 128, 10, 2, 128), f)
    for c in range(NC_D):
        wg = dwc[c * 128:(c + 1) * 128] * g1[c * 128:(c + 1) * 128, None]
        for q in range(5):
            t0, t1 = TAP_PAIRS[q]
            np.einsum('ii->i', cdx[c, :, q, 0])[:] = wg[:, t0]
            np.einsum('ii->i', cdx[c, :, q + 5, 0])[:] = -wg[:, t0]
            if t1 is not None:
                np.einsum('ii->i', cdx[c, :, q, 1])[:] = wg[:, t1]
                np.einsum('ii->i', cdx[c, :, q + 5, 1])[:] = -wg[:, t1]

    # conv bias -> GRU sigmoid bias: hgb_d = dwc_b @ gru_w_d
    w1 = np.asarray(gru1_w, f)
    w2 = np.asarray(gru2_w, f)
    db = np.asarray(dwc_b, f)
    hgb1 = db @ w1
    hgb2 = db @ w2
    colmat = lambda v, n: np.ascontiguousarray(v.reshape(n, 128).T)
    g1c = _kchunk(w1, 4).astype(np.float32)
    g2c = _kchunk(w2, 4).astype(np.float32)
    g1c[0, 3, :] = hgb1
    g2c[0, 3, :] = hgb2

    # gamma2/beta2 fold into p1w/pb1; pm2 row in K-chunk 3
    p1e = np.asarray(p1_w, f) * np.asarray(gamma2, f)[:, None]
    pb1e = np.asarray(p1_b, f) + np.asarray(beta2, f) @ np.asarray(p1_w, f)
    p1c = _kchunk(p1e, 4).astype(np.float32)
    p1c[0, 3, :] = -p1e.sum(axis=0)
    return dict(
        g1w=g1c.astype(F8), g2w=g2c.astype(F8),
        g1o=_kchunk16(np.asarray(gru1_out, f), NC_H),
        g2o=_kchunk16(np.asarray(gru2_out, f), NC_H),
        p1w=p1c.astype(F8), p2w=_kchunk(np.asarray(p2_w, f), NC_M),
        cdx=cdx.astype(F8),
        ident=np.eye(128, dtype=np.float16),
        pb1=colmat(pb1e, NC_M), pb2=colmat(np.asarray(p2_b, f), NC_D),
    )


def kernel(x, gamma1, beta1, dwc_w, dwc_b, gru1_w, gru1_out, gru2_w, gru2_out,
           gamma2, beta2, p1_w, p1_b, p2_w, p2_b, h, w):
    assert np.allclose(np.asarray(beta1), 0.0), "beta1 fold not implemented"
    x = np.asarray(x, np.float32)
    nc = _get_nc()
    xT = np.ascontiguousarray(x.transpose(0, 2, 1)).astype(np.float16)
    wmap = make_weight_maps(gamma1, beta1, dwc_w, dwc_b, gru1_w, gru1_out,
                            gru2_w, gru2_out, gamma2, beta2, p1_w, p1_b,
                            p2_w, p2_b)
    in_maps = []
    for i in range(N_CORES):
        m = dict(wmap)
        m["xT"] = xT[i * NS:(i + 1) * NS]
        in_maps.append(m)
    res = run_bass_kernel_spmd(nc, in_maps, list(range(N_CORES)))
    yT = np.concatenate([res.results[i]["y"] for i in range(N_CORES)], axis=0)
    return np.ascontiguousarray(
        yT.astype(np.float32).transpose(0, 2, 1))
